# revision 23
# baseline (speedup 1.0000x reference)
"""Perona-Malik anisotropic diffusion (option 2), 10 iterations, on 8 TRN2 NeuronCores.

Pure data parallel: each core takes 2 of the 16 batch images (= 6 channel-images
of 512x512).  Per core, u is held in SBUF as 6 per-image tiles
[128 rows, 4 bands x 520 cols] bf16 (interior at cols 2..513, zero pads both
sides), updated in place.

No shifted copies of u are materialized.  Instead the stencil is decomposed
into two difference fields per band:
    dN(c) = u(r-1,c) - u(r,c)   (PSUM, one folded (S_UP - I) matmul + seams)
    dS(c) = u(r+1,c) - u(r,c)   (PSUM, (S_DN - I) matmul + seams)
    dE(j) = u(r,j) - u(r,j-1)   (SBUF bf16, GPSIMD/VectorE tensor_sub)
and every diagonal difference telescopes:  e.g.
    u(r-1,c+1) - u(r,c) = dN(c+1) + dE(c+1),   u(r-1,c-1)-u(r,c) = dN(c-1)-dE(c).

Division of labor per iteration (per core):
  - TensorEngine: dN/dS builds in [128,1028] PSUM tiles (2 bands per tile,
    col k of a band region = d@(k-1); edge columns auto-zero because the rhs
    slices start in u's zero pad columns; band-seam rows via one-hot matmuls
    from a DMA-maintained row-127 shadow for dN, and from the next band's
    partition 0 for dS), plus the accumulation
        upd = 7*u + sum_k y_k   (1/DT folded in).
  - ScalarEngine: N, S, E, W directions as single Tanh passes
        w c(nab) nab ~= (kappa/sqrt3) * tanh(sqrt3 (f d + b)/kappa)
    (N/S from the PSUM windows, E/W from the SBUF dE field), and the update
    u_{t+1} = DT * upd (PSUM -> bf16 SBUF, in place).
  - VectorEngine: the four diagonals with fused custom DVE ops per 2-band
    window:  y = v(1-v^2)^2,  v = (in0 +- in1)*s0 + s1  with in0 = dN/dS
    (PSUM) and in1 = dE (SBUF), approximating w c(nab) nab with
    L ~= kappa*sqrt(2); plus dE for one image.
  - GPSIMD: dE tensor_sub for the other five images.

biases/factors are folded into immediates at trace time (the kernel is compiled
inside kernel(), cached on the biases/factors bytes).  Numpy-simulated max rel
err of this scheme vs the exact f32 reference: ~4.8e-3 (gate 2e-2).
"""
import math
import os
import sys

import numpy as np

for _p in ("/root/.axon_site", "/root/.axon_site/_ro/trn_rl_repo", "/opt/trn_rl_repo"):
    if os.path.isdir(_p) and _p not in sys.path:
        sys.path.append(_p)

import concourse.bass as bass
import concourse.tile as tile
from concourse import bacc, mybir
from concourse.bass_utils import run_bass_kernel_spmd

# ---------------- problem constants (hardcoded; kernel.py is self-contained) ---
B, C, H, W = 16, 3, 512, 512
NUM_ITER = 10
DT = 1.0 / 7.0
KAPPA = 30.0
DIR_W = [1.0, 1.0, 1.0, 1.0, 0.5, 0.5, 0.5, 0.5]

N_CORES = 8
IMGS = (B // N_CORES) * C          # 6 images per core
BANDS = H // 128                   # 4 bands per image
WP = 520                           # padded band width (interior at cols 2..513)
TW = BANDS * WP                    # image tile width 2080
WE = 514                           # dE field band width (col j = dE@j, j=0..513)
DW = 514                           # d-field band region width (col k = d@(k-1))
SQ3K = math.sqrt(3.0) / KAPPA
GP_IMGS = 5                        # images whose dE is computed on GPSIMD

BF16 = mybir.dt.bfloat16
F32 = mybir.dt.float32

# ---------------- custom DVE ops: fused quintic diffusion directions -----------
from concourse.dve_spec import Spec, Src0, Src1, One, sq, lower
from concourse.dve_ops import (
    OPS,
    DveOp,
    _SUB_OPCODE_FOR_NAME,
    _CUSTOM_DVE_ROW_BASE,
    C0,
    C1,
)
from concourse.dve_uop import DveOpSpec

# out = v*(1 - v^2)^2,  v = (in0 -+ in1)*s0 + s1
# with s0 = w*f/(L/2), s1 = w*b/(L/2) and L ~= w*kappa*sqrt(2) so that
# v^2 ~= ((f d + b)/kappa)^2 / 2; the (L/2)-scaled accumulation matmul restores
# the output scale.  Approximates w nab / (1 + (nab/kappa)^2).
PM_EW_L = 42.5


def _pm_sub_ref(in0, in1, s0, s1, imm2):
    v = (in0.astype(np.float32) - in1.astype(np.float32)) * s0 + s1
    g = 1.0 - v * v
    return v * (g * g)


def _pm_add_ref(in0, in1, s0, s1, imm2):
    v = (in0.astype(np.float32) + in1.astype(np.float32)) * s0 + s1
    g = 1.0 - v * v
    return v * (g * g)


def _register_pm_op(name, combine, ref):
    if name in _SUB_OPCODE_FOR_NAME:
        return next(op for op in OPS if op.name == name)
    _v = combine * C0 + C1
    _g = One - sq(_v)
    spec = Spec(body=_v * sq(_g), reference=ref)
    row = _CUSTOM_DVE_ROW_BASE + len(OPS)
    _SUB_OPCODE_FOR_NAME[name] = row
    shas = {}
    for ver in ("v3", "v4"):
        sp = DveOpSpec(name=name, opcode=row, uops=lower(spec, ver=ver), rd1_en=True)
        shas[ver] = sp.sha(ver)
    op = DveOp(name, spec, subdim=False, uops_sha=shas)
    OPS.append(op)
    return op


PM_SUB_OP = _register_pm_op("PM_DIFFUSE_EW_ANT", Src0 - Src1, _pm_sub_ref)
PM_ADD_OP = _register_pm_op("PM_DIFFUSE_ADD_ANT", Src0 + Src1, _pm_add_ref)


# ---------------- weight matrices for TensorE ---------------------------------
def _weight_mats():
    import ml_dtypes

    I = np.eye(128, dtype=np.float32)
    S_upT = np.zeros((128, 128), np.float32)   # out[m] = u[m-1]
    S_upT[np.arange(127), np.arange(1, 128)] = 1.0
    S_dnT = np.zeros((128, 128), np.float32)   # out[m] = u[m+1]
    S_dnT[np.arange(1, 128), np.arange(127)] = 1.0
    E_dnT = np.zeros((128, 128), np.float32)   # out[127] = next[0]
    E_dnT[0, 127] = 1.0
    U_upT = np.zeros((128, 128), np.float32)   # out[0] = prev[127]
    U_upT[127, 0] = 1.0
    I7 = 7.0 * I                       # folds 1/DT into the u term
    T1 = (KAPPA / math.sqrt(3.0)) * I  # tanh-direction output scale (w=1)
    IL2 = (PM_EW_L / 2) * I            # custom-op output scale (w=0.5 diagonals)
    DN_M = S_upT - I                   # d_N = u[m-1] - u[m]
    DS_M = S_dnT - I                   # d_S = u[m+1] - u[m]
    return np.stack([I7, T1, IL2, DN_M, DS_M, U_upT, E_dnT]).astype(ml_dtypes.bfloat16)


# ---------------- kernel build -------------------------------------------------
def build_nc(biases: np.ndarray, factors: np.ndarray):
    """Trace the full 10-iteration kernel; biases/factors folded as immediates."""
    biases = np.asarray(biases, np.float32)
    factors = np.asarray(factors, np.float32)

    ACT_TANH = mybir.ActivationFunctionType.Tanh
    ACT_COPY = mybir.ActivationFunctionType.Copy
    nc = bacc.Bacc()
    x_d = nc.declare_dram_parameter("x", [IMGS, H, W], F32, isOutput=False)
    w_d = nc.declare_dram_parameter("wmat", [7, 128, 128], BF16, isOutput=False)
    o_d = nc.declare_dram_parameter("out", [IMGS, H, W], F32, isOutput=True)

    import dataclasses as _dc
    from contextlib import ExitStack

    with tile.TileContext(nc) as tc:
        with ExitStack() as ctx:
            upool = ctx.enter_context(tc.tile_pool(name="u", bufs=1))
            wpool = ctx.enter_context(tc.tile_pool(name="w", bufs=1))
            io_pool = ctx.enter_context(tc.tile_pool(name="io", bufs=4))
            spool = ctx.enter_context(tc.tile_pool(name="s", bufs=2))
            dn_pool = ctx.enter_context(tc.tile_pool(name="dn", bufs=1, space="PSUM"))
            ds_pool = ctx.enter_context(tc.tile_pool(name="ds", bufs=1, space="PSUM"))
            upd_pool = ctx.enter_context(tc.tile_pool(name="upd", bufs=2, space="PSUM"))

            def view(base_ap, stride, count, n):
                """sliced AP -> add outer dim: [p, count, n] at given stride."""
                return _dc.replace(
                    base_ap, ap=[base_ap.ap[0], [stride, count], [1, n]]
                )

            # persistent tiles
            wt = [wpool.tile([128, 128], BF16, tag=f"w{i}", name=f"w{i}") for i in range(7)]
            IDENT7, T1_M, IL2_M, DN_M, DS_M, U_UP, E_DN = wt
            uA = [upool.tile([128, TW], BF16, tag=f"uA{i}", name=f"uA{i}") for i in range(IMGS)]

            for i in range(7):
                nc.sync.dma_start(wt[i][:], w_d[i])

            # tanh bias scalars for the four axial directions, per channel
            bias_ap = {}
            for k in range(4):
                for c in range(C):
                    bt = wpool.tile([128, 1], F32, tag=f"b{k}_{c}", name=f"b{k}_{c}")
                    nc.gpsimd.memset(bt[:], float(SQ3K * biases[k, c]))
                    bias_ap[(k, c)] = bt

            # load input: DMA f32 -> staging, convert to bf16 interior; zero pads
            for img in range(IMGS):
                nc.gpsimd.memset(uA[img][:], 0.0)
                for jb in range(BANDS):
                    st = io_pool.tile([128, W], F32, tag="stage_in")
                    nc.sync.dma_start(st[:], x_d[img, jb * 128 : (jb + 1) * 128, :])
                    nc.scalar.copy(uA[img][:, jb * WP + 2 : jb * WP + 2 + W], st[:])

            # d-field region splits within a [128,1028] window tile: band A at
            # cols 0..513, band B at 514..1027; matmul outs must not cross the
            # PSUM bank boundaries at cols 512 and 1024.
            A_SPLITS = [(0, 512), (512, 514)]       # (tile col start, end)
            B_SPLITS = [(514, 1024), (1024, 1028)]

            def eval_img(t, img):
                if True:
                    ch = img % C
                    u = uA[img]

                    # ---- dE[j] = u@j - u@(j-1) (SBUF bf16, GPSIMD) ----
                    dE = spool.tile([128, BANDS * WE], BF16, tag="dE", name="dE")
                    nc.gpsimd.tensor_sub(
                        view(dE[:, 0:WE], WE, BANDS, WE),
                        view(u[:, 2 : 2 + WE], WP, BANDS, WE),
                        view(u[:, 1 : 1 + WE], WP, BANDS, WE),
                    )

                    # ---- ScalarE: tanh E, W from dE (whole image) ----
                    # y_E[c] = (k/sqrt3) tanh(+a_E dE@(c+1) + c_E)
                    # y_W[c] = (k/sqrt3) tanh(-a_W dE@c     + c_W)
                    sE = spool.tile([128, BANDS * W], BF16, tag="sE", name="sE")
                    sW = spool.tile([128, BANDS * W], BF16, tag="sW", name="sW")
                    nc.scalar.activation(
                        view(sE[:, 0:W], W, BANDS, W),
                        view(dE[:, 1 : 1 + W], WE, BANDS, W),
                        ACT_TANH,
                        scale=float(SQ3K * factors[3, ch]),
                        bias=bias_ap[(3, ch)][:],
                    )
                    nc.scalar.activation(
                        view(sW[:, 0:W], W, BANDS, W),
                        view(dE[:, 0:W], WE, BANDS, W),
                        ACT_TANH,
                        scale=float(-SQ3K * factors[2, ch]),
                        bias=bias_ap[(2, ch)][:],
                    )

                    sN = spool.tile([128, BANDS * W], BF16, tag="sN", name="sN")
                    sS = spool.tile([128, BANDS * W], BF16, tag="sS", name="sS")
                    ydiag = {k: spool.tile([128, BANDS * W], BF16, tag=f"yD{k}", name=f"yD{k}")
                             for k in (4, 5, 6, 7)}

                    def emit_window(w2):
                        b0 = 2 * w2
                        dN = dn_pool.tile([128, 2 * DW], F32, name="dN")
                        dS = ds_pool.tile([128, 2 * DW], F32, name="dS")
                        for splits, band in ((A_SPLITS, b0), (B_SPLITS, b0 + 1)):
                            ub = band * WP
                            rb = splits[0][0]
                            for s, e in splits:
                                wdt = e - s
                                rs = ub + 1 + (s - rb)
                                # dN = (S_UP - I) @ u; rhs starts in u's zero pad
                                nc.tensor.matmul(
                                    dN[:, s:e], DN_M[:], u[:, rs : rs + wdt],
                                    start=True, stop=(band == 0),
                                )
                                if band > 0:
                                    # row 0 += u_prev_band[127] (one-hot 127->0;
                                    # contraction reads partition 127 freely)
                                    ps = (band - 1) * WP + 1 + (s - rb)
                                    nc.tensor.matmul(
                                        dN[:, s:e], U_UP[:],
                                        u[:, ps : ps + wdt],
                                        start=False, stop=True,
                                    )
                                nc.tensor.matmul(
                                    dS[:, s:e], DS_M[:], u[:, rs : rs + wdt],
                                    start=True, stop=(band == 3),
                                )
                                if band < 3:
                                    # row 127 += u_next_band[0] (one-hot on P0)
                                    ns = (band + 1) * WP + 1 + (s - rb)
                                    nc.tensor.matmul(
                                        dS[:, s:e], E_DN[:],
                                        u[:, ns : ns + wdt],
                                        start=False, stop=True,
                                    )

                        # ---- ScalarE: tanh N, S over the window ----
                        osl = slice(w2 * 1024, (w2 + 1) * 1024)
                        nc.scalar.activation(
                            view(sN[:, osl][:, 0:W], W, 2, W),
                            view(dN[:, 1 : 1 + W], DW, 2, W),
                            ACT_TANH,
                            scale=float(SQ3K * factors[0, ch]),
                            bias=bias_ap[(0, ch)][:],
                        )
                        nc.scalar.activation(
                            view(sS[:, osl][:, 0:W], W, 2, W),
                            view(dS[:, 1 : 1 + W], DW, 2, W),
                            ACT_TANH,
                            scale=float(SQ3K * factors[1, ch]),
                            bias=bias_ap[(1, ch)][:],
                        )

                        # ---- VectorE: fused quintic diagonals over the window --
                        # NE: dN@(c+1)+dE@(c+1)  SE: dS@(c+1)+dE@(c+1)
                        # SW: dS@(c-1)-dE@c      NW: dN@(c-1)-dE@c
                        eb = w2 * 2 * WE
                        # dN readers first so the dN buffer frees earlier
                        for k, (op, din, dcol, ecol) in {
                            4: (PM_ADD_OP, dN, 2, 1),
                            7: (PM_SUB_OP, dN, 0, 0),
                            5: (PM_ADD_OP, dS, 2, 1),
                            6: (PM_SUB_OP, dS, 0, 0),
                        }.items():
                            nc.vector._custom_dve(
                                op,
                                out=view(ydiag[k][:, osl][:, 0:W], W, 2, W),
                                in0=view(din[:, dcol : dcol + W], DW, 2, W),
                                in1=view(dE[:, eb + ecol : eb + ecol + W], WE, 2, W),
                                s0=float(DIR_W[k] * factors[k, ch] / (PM_EW_L / 2)),
                                s1=float(DIR_W[k] * biases[k, ch] / (PM_EW_L / 2)),
                            )

                    emit_window(0)
                    yield None
                    emit_window(1)
                    # pre-sum one diagonal pair on GPSIMD (one fewer accum
                    # matmul per band; safe now that emission is pipelined)
                    y45 = spool.tile([128, BANDS * W], BF16, tag="y45", name="y45")
                    nc.gpsimd.tensor_add(y45[:], ydiag[4][:], ydiag[5][:])

                    yield (sN, sS, sE, sW, ydiag, y45)

            def accum_bands(t, img, tiles, bands):
                sN, sS, sE, sW, ydiag, y45 = tiles
                u = uA[img]
                # ---- TensorE: upd = 7u + sum_k y_k; ScalarE: u' = DT*upd ----
                for b in bands:
                    cl = b * WP + 2
                    sl = slice(b * W, (b + 1) * W)
                    upd = upd_pool.tile([128, W], F32, name="upd")
                    nc.tensor.matmul(upd[:], IDENT7[:], u[:, cl : cl + W],
                                     start=True, stop=False)
                    for s in (sN, sS, sE, sW):
                        nc.tensor.matmul(upd[:], T1_M[:], s[:, sl],
                                         start=False, stop=False)
                    nc.tensor.matmul(upd[:], IL2_M[:], y45[:, sl],
                                     start=False, stop=False)
                    for i, k in enumerate((6, 7)):
                        nc.tensor.matmul(upd[:], IL2_M[:], ydiag[k][:, sl],
                                         start=False, stop=(i == 1))
                    if t < NUM_ITER - 1:
                        nc.scalar.activation(
                            u[:, cl : cl + W], upd[:], ACT_COPY, scale=float(DT),
                        )
                    else:
                        so = io_pool.tile([128, W], F32, tag="stage_out", name="so")
                        nc.scalar.activation(so[:], upd[:], ACT_COPY, scale=float(DT))
                        nc.sync.dma_start(
                            o_d[img, b * 128 : (b + 1) * 128, :], so[:]
                        )

            # Software pipeline: emit image i's accumulation interleaved into
            # image i+1's eval (half between the two windows) so each engine's
            # in-order stream always has independent work behind a stall.
            for t in range(NUM_ITER):
                pending = None
                for img in range(IMGS):
                    g = eval_img(t, img)
                    next(g)
                    if pending is not None:
                        accum_bands(t, img - 1, pending, (0, 1))
                    tiles = next(g)
                    if pending is not None:
                        accum_bands(t, img - 1, pending, (2, 3))
                    pending = tiles
                accum_bands(t, IMGS - 1, pending, range(BANDS))

    nc.finalize()
    return nc


def _install_ntff_hook():
    """The agent image's antenv lacks axon_hooks; recreate it so trace=True works."""
    import types

    try:
        from antenv.axon_hooks import get_axon_ntff_profile_hook  # noqa: F401

        return
    except ImportError:
        pass
    import antenv

    mod = types.ModuleType("antenv.axon_hooks")
    _state = {"hook": None}
    mod.set_axon_ntff_profile_hook = lambda h: _state.__setitem__("hook", h)
    mod.get_axon_ntff_profile_hook = lambda: _state["hook"]
    sys.modules["antenv.axon_hooks"] = mod
    antenv.axon_hooks = mod
    so_path = "/opt/axon/libaxon_pjrt.so"
    if os.path.exists(so_path):
        sys.path.insert(0, "/root/.axon_site")
        try:
            from trn_agent_boot.trn_boot import _ntff_profile_via_ctypes

            hook = _ntff_profile_via_ctypes(so_path)
            if hook is not None:
                mod.set_axon_ntff_profile_hook(hook)
        except Exception as e:
            print(f"ntff hook install failed: {e}")


_CACHE = {}


def _get_nc(biases, factors):
    key = (biases.tobytes(), factors.tobytes())
    if key not in _CACHE:
        _CACHE[key] = build_nc(biases, factors)
    return _CACHE[key]


def kernel(x, biases, factors, _trace=False):
    x = np.ascontiguousarray(np.asarray(x, np.float32))
    biases = np.asarray(biases, np.float32)
    factors = np.asarray(factors, np.float32)
    nc = _get_nc(biases, factors)
    if _trace:
        _install_ntff_hook()

    wmat = _weight_mats()
    per_core = B // N_CORES
    in_maps = [
        {
            "x": x[i * per_core : (i + 1) * per_core].reshape(IMGS, H, W),
            "wmat": wmat,
        }
        for i in range(N_CORES)
    ]
    res = run_bass_kernel_spmd(nc, in_maps, core_ids=list(range(N_CORES)), trace=_trace)
    out = np.concatenate(
        [res.results[i]["out"].reshape(per_core, C, H, W) for i in range(N_CORES)],
        axis=0,
    )
    if _trace:
        kernel.last_exec_time_ns = res.exec_time_ns
        kernel.last_results = res
    return out


# revision 26
# speedup vs baseline: 1.2392x; 1.2392x over previous
"""Perona-Malik anisotropic diffusion (option 2), 10 iterations, on 8 TRN2 NeuronCores.

Pure data parallel: each core takes 2 of the 16 batch images (= 6 channel-images
of 512x512).  Per core, u is held in SBUF as 6 per-image tiles
[128 rows, 4 bands x 520 cols] bf16 (interior at cols 2..513, zero pads both
sides), updated in place.

No shifted copies of u are materialized.  Instead the stencil is decomposed
into two difference fields per band:
    dN(c) = u(r-1,c) - u(r,c)   (PSUM, one folded (S_UP - I) matmul + seams)
    dS(c) = u(r+1,c) - u(r,c)   (PSUM, (S_DN - I) matmul + seams)
    dE(j) = u(r,j) - u(r,j-1)   (SBUF bf16, GPSIMD/VectorE tensor_sub)
and every diagonal difference telescopes:  e.g.
    u(r-1,c+1) - u(r,c) = dN(c+1) + dE(c+1),   u(r-1,c-1)-u(r,c) = dN(c-1)-dE(c).

Division of labor per iteration (per core):
  - TensorEngine: dN/dS builds in [128,1028] PSUM tiles (2 bands per tile,
    col k of a band region = d@(k-1); edge columns auto-zero because the rhs
    slices start in u's zero pad columns; band-seam rows via one-hot matmuls
    from a DMA-maintained row-127 shadow for dN, and from the next band's
    partition 0 for dS), plus the accumulation
        upd = 7*u + sum_k y_k   (1/DT folded in).
  - ScalarEngine: N, S, E, W directions as single Tanh passes
        w c(nab) nab ~= (kappa/sqrt3) * tanh(sqrt3 (f d + b)/kappa)
    (N/S from the PSUM windows, E/W from the SBUF dE field), and the update
    u_{t+1} = DT * upd (PSUM -> bf16 SBUF, in place).
  - VectorEngine: the four diagonals with fused custom DVE ops per 2-band
    window:  y = v(1-v^2)^2,  v = (in0 +- in1)*s0 + s1  with in0 = dN/dS
    (PSUM) and in1 = dE (SBUF), approximating w c(nab) nab with
    L ~= kappa*sqrt(2); plus dE for one image.
  - GPSIMD: dE tensor_sub for the other five images.

biases/factors are folded into immediates at trace time (the kernel is compiled
inside kernel(), cached on the biases/factors bytes).  Numpy-simulated max rel
err of this scheme vs the exact f32 reference: ~4.8e-3 (gate 2e-2).
"""
import math
import os
import sys

import numpy as np

for _p in ("/root/.axon_site", "/root/.axon_site/_ro/trn_rl_repo", "/opt/trn_rl_repo"):
    if os.path.isdir(_p) and _p not in sys.path:
        sys.path.append(_p)

import concourse.bass as bass
import concourse.tile as tile
from concourse import bacc, mybir
from concourse.bass_utils import run_bass_kernel_spmd

# ---------------- problem constants (hardcoded; kernel.py is self-contained) ---
B, C, H, W = 16, 3, 512, 512
NUM_ITER = 10
DT = 1.0 / 7.0
KAPPA = 30.0
DIR_W = [1.0, 1.0, 1.0, 1.0, 0.5, 0.5, 0.5, 0.5]

N_CORES = 8
IMGS = (B // N_CORES) * C          # 6 images per core
BANDS = H // 128                   # 4 bands per image
WP = 520                           # padded band width (interior at cols 2..513)
TW = BANDS * WP                    # image tile width 2080
WE = 514                           # dE field band width (col j = dE@j, j=0..513)
DW = 514                           # d-field band region width (col k = d@(k-1))
SQ3K = math.sqrt(3.0) / KAPPA
GP_IMGS = 5                        # images whose dE is computed on GPSIMD

BF16 = mybir.dt.bfloat16
F32 = mybir.dt.float32

# ---------------- custom DVE ops: fused quintic diffusion directions -----------
from concourse.dve_spec import Spec, Src0, Src1, One, sq, lower
from concourse.dve_ops import (
    OPS,
    DveOp,
    _SUB_OPCODE_FOR_NAME,
    _CUSTOM_DVE_ROW_BASE,
    C0,
    C1,
)
from concourse.dve_uop import DveOpSpec

# out = v*(1 - v^2)^2,  v = (in0 -+ in1)*s0 + s1
# with s0 = w*f/(L/2), s1 = w*b/(L/2) and L ~= w*kappa*sqrt(2) so that
# v^2 ~= ((f d + b)/kappa)^2 / 2; the (L/2)-scaled accumulation matmul restores
# the output scale.  Approximates w nab / (1 + (nab/kappa)^2).
PM_EW_L = 42.5


def _pm_sub_ref(in0, in1, s0, s1, imm2):
    v = (in0.astype(np.float32) - in1.astype(np.float32)) * s0 + s1
    g = 1.0 - v * v
    return v * (g * g)


def _pm_add_ref(in0, in1, s0, s1, imm2):
    v = (in0.astype(np.float32) + in1.astype(np.float32)) * s0 + s1
    g = 1.0 - v * v
    return v * (g * g)


def _register_pm_op(name, combine, ref):
    if name in _SUB_OPCODE_FOR_NAME:
        return next(op for op in OPS if op.name == name)
    _v = combine * C0 + C1
    _g = One - sq(_v)
    spec = Spec(body=_v * sq(_g), reference=ref)
    row = _CUSTOM_DVE_ROW_BASE + len(OPS)
    _SUB_OPCODE_FOR_NAME[name] = row
    shas = {}
    for ver in ("v3", "v4"):
        sp = DveOpSpec(name=name, opcode=row, uops=lower(spec, ver=ver), rd1_en=True)
        shas[ver] = sp.sha(ver)
    op = DveOp(name, spec, subdim=False, uops_sha=shas)
    OPS.append(op)
    return op


PM_SUB_OP = _register_pm_op("PM_DIFFUSE_EW_ANT", Src0 - Src1, _pm_sub_ref)
PM_ADD_OP = _register_pm_op("PM_DIFFUSE_ADD_ANT", Src0 + Src1, _pm_add_ref)


# ---------------- weight matrices for TensorE ---------------------------------
def _weight_mats():
    import ml_dtypes

    I = np.eye(128, dtype=np.float32)
    S_upT = np.zeros((128, 128), np.float32)   # out[m] = u[m-1]
    S_upT[np.arange(127), np.arange(1, 128)] = 1.0
    S_dnT = np.zeros((128, 128), np.float32)   # out[m] = u[m+1]
    S_dnT[np.arange(1, 128), np.arange(127)] = 1.0
    E_dnT = np.zeros((128, 128), np.float32)   # out[127] = next[0]
    E_dnT[0, 127] = 1.0
    U_upT = np.zeros((128, 128), np.float32)   # out[0] = prev[127]
    U_upT[127, 0] = 1.0
    I7 = 7.0 * I                       # folds 1/DT into the u term
    T1 = (KAPPA / math.sqrt(3.0)) * I  # tanh-direction output scale (w=1)
    IL2 = (PM_EW_L / 2) * I            # custom-op output scale (w=0.5 diagonals)
    DN_M = S_upT - I                   # d_N = u[m-1] - u[m]
    DS_M = S_dnT - I                   # d_S = u[m+1] - u[m]
    return np.stack([I7, T1, IL2, DN_M, DS_M, U_upT, E_dnT]).astype(ml_dtypes.bfloat16)


# ---------------- kernel build -------------------------------------------------
def build_nc(biases: np.ndarray, factors: np.ndarray):
    """Trace the full 10-iteration kernel; biases/factors folded as immediates."""
    biases = np.asarray(biases, np.float32)
    factors = np.asarray(factors, np.float32)

    ACT_TANH = mybir.ActivationFunctionType.Tanh
    ACT_COPY = mybir.ActivationFunctionType.Copy
    nc = bacc.Bacc()
    x_d = nc.declare_dram_parameter("x", [IMGS, H, W], F32, isOutput=False)
    w_d = nc.declare_dram_parameter("wmat", [7, 128, 128], BF16, isOutput=False)
    o_d = nc.declare_dram_parameter("out", [IMGS, H, W], F32, isOutput=True)

    import dataclasses as _dc
    from contextlib import ExitStack

    with tile.TileContext(nc) as tc:
        with ExitStack() as ctx:
            upool = ctx.enter_context(tc.tile_pool(name="u", bufs=1))
            wpool = ctx.enter_context(tc.tile_pool(name="w", bufs=1))
            io_pool = ctx.enter_context(tc.tile_pool(name="io", bufs=4))
            spool = ctx.enter_context(tc.tile_pool(name="s", bufs=2))
            dn_pool = ctx.enter_context(tc.tile_pool(name="dn", bufs=1, space="PSUM"))
            ds_pool = ctx.enter_context(tc.tile_pool(name="ds", bufs=1, space="PSUM"))
            upd_pool = ctx.enter_context(tc.tile_pool(name="upd", bufs=2, space="PSUM"))

            def view(base_ap, stride, count, n):
                """sliced AP -> add outer dim: [p, count, n] at given stride."""
                return _dc.replace(
                    base_ap, ap=[base_ap.ap[0], [stride, count], [1, n]]
                )

            # persistent tiles
            wt = [wpool.tile([128, 128], BF16, tag=f"w{i}", name=f"w{i}") for i in range(7)]
            IDENT7, T1_M, IL2_M, DN_M, DS_M, U_UP, E_DN = wt
            uA = [upool.tile([128, TW], BF16, tag=f"uA{i}", name=f"uA{i}") for i in range(IMGS)]

            for i in range(7):
                nc.sync.dma_start(wt[i][:], w_d[i])

            # tanh bias scalars for the four axial directions, per channel
            bias_ap = {}
            for k in range(4):
                for c in range(C):
                    bt = wpool.tile([128, 1], F32, tag=f"b{k}_{c}", name=f"b{k}_{c}")
                    nc.gpsimd.memset(bt[:], float(SQ3K * biases[k, c]))
                    bias_ap[(k, c)] = bt

            # load input: DMA f32 -> staging, convert to bf16 interior; zero pads
            for img in range(IMGS):
                nc.gpsimd.memset(uA[img][:], 0.0)
                for jb in range(BANDS):
                    st = io_pool.tile([128, W], F32, tag="stage_in")
                    nc.sync.dma_start(st[:], x_d[img, jb * 128 : (jb + 1) * 128, :])
                    nc.scalar.copy(uA[img][:, jb * WP + 2 : jb * WP + 2 + W], st[:])

            # d-field region splits within a [128,1028] window tile: band A at
            # cols 0..513, band B at 514..1027; matmul outs must not cross the
            # PSUM bank boundaries at cols 512 and 1024.
            A_SPLITS = [(0, 512), (512, 514)]       # (tile col start, end)
            B_SPLITS = [(514, 1024), (1024, 1028)]

            def eval_img(t, img):
                if True:
                    ch = img % C
                    u = uA[img]

                    # ---- dE[j] = u@j - u@(j-1) (SBUF bf16, GPSIMD) ----
                    dE = spool.tile([128, BANDS * WE], BF16, tag="dE", name="dE")
                    nc.gpsimd.tensor_sub(
                        view(dE[:, 0:WE], WE, BANDS, WE),
                        view(u[:, 2 : 2 + WE], WP, BANDS, WE),
                        view(u[:, 1 : 1 + WE], WP, BANDS, WE),
                    )

                    # ---- ScalarE: tanh E, W from dE (whole image) ----
                    # y_E[c] = (k/sqrt3) tanh(+a_E dE@(c+1) + c_E)
                    # y_W[c] = (k/sqrt3) tanh(-a_W dE@c     + c_W)
                    sE = spool.tile([128, BANDS * W], BF16, tag="sE", name="sE")
                    sW = spool.tile([128, BANDS * W], BF16, tag="sW", name="sW")
                    nc.scalar.activation(
                        view(sE[:, 0:W], W, BANDS, W),
                        view(dE[:, 1 : 1 + W], WE, BANDS, W),
                        ACT_TANH,
                        scale=float(SQ3K * factors[3, ch]),
                        bias=bias_ap[(3, ch)][:],
                    )
                    nc.scalar.activation(
                        view(sW[:, 0:W], W, BANDS, W),
                        view(dE[:, 0:W], WE, BANDS, W),
                        ACT_TANH,
                        scale=float(-SQ3K * factors[2, ch]),
                        bias=bias_ap[(2, ch)][:],
                    )

                    sN = spool.tile([128, BANDS * W], BF16, tag="sN", name="sN")
                    sS = spool.tile([128, BANDS * W], BF16, tag="sS", name="sS")
                    ydiag = {k: spool.tile([128, BANDS * W], BF16, tag=f"yD{k}", name=f"yD{k}")
                             for k in (4, 5, 6, 7)}

                    def emit_window(w2):
                        b0 = 2 * w2
                        dN = dn_pool.tile([128, 2 * DW], F32, name="dN")
                        dS = ds_pool.tile([128, 2 * DW], F32, name="dS")
                        for splits, band in ((A_SPLITS, b0), (B_SPLITS, b0 + 1)):
                            ub = band * WP
                            rb = splits[0][0]
                            for s, e in splits:
                                wdt = e - s
                                rs = ub + 1 + (s - rb)
                                # dN = (S_UP - I) @ u; rhs starts in u's zero pad
                                nc.tensor.matmul(
                                    dN[:, s:e], DN_M[:], u[:, rs : rs + wdt],
                                    start=True, stop=(band == 0),
                                )
                                if band > 0:
                                    # row 0 += u_prev_band[127] (one-hot 127->0;
                                    # contraction reads partition 127 freely)
                                    ps = (band - 1) * WP + 1 + (s - rb)
                                    nc.tensor.matmul(
                                        dN[:, s:e], U_UP[:],
                                        u[:, ps : ps + wdt],
                                        start=False, stop=True,
                                    )
                                nc.tensor.matmul(
                                    dS[:, s:e], DS_M[:], u[:, rs : rs + wdt],
                                    start=True, stop=(band == 3),
                                )
                                if band < 3:
                                    # row 127 += u_next_band[0] (one-hot on P0)
                                    ns = (band + 1) * WP + 1 + (s - rb)
                                    nc.tensor.matmul(
                                        dS[:, s:e], E_DN[:],
                                        u[:, ns : ns + wdt],
                                        start=False, stop=True,
                                    )

                        # ---- ScalarE: tanh N, S over the window ----
                        osl = slice(w2 * 1024, (w2 + 1) * 1024)
                        nc.scalar.activation(
                            view(sN[:, osl][:, 0:W], W, 2, W),
                            view(dN[:, 1 : 1 + W], DW, 2, W),
                            ACT_TANH,
                            scale=float(SQ3K * factors[0, ch]),
                            bias=bias_ap[(0, ch)][:],
                        )
                        nc.scalar.activation(
                            view(sS[:, osl][:, 0:W], W, 2, W),
                            view(dS[:, 1 : 1 + W], DW, 2, W),
                            ACT_TANH,
                            scale=float(SQ3K * factors[1, ch]),
                            bias=bias_ap[(1, ch)][:],
                        )

                        # ---- VectorE: fused quintic diagonals over the window --
                        # NE: dN@(c+1)+dE@(c+1)  SE: dS@(c+1)+dE@(c+1)
                        # SW: dS@(c-1)-dE@c      NW: dN@(c-1)-dE@c
                        eb = w2 * 2 * WE
                        # dN readers first so the dN buffer frees earlier
                        for k, (op, din, dcol, ecol) in {
                            4: (PM_ADD_OP, dN, 2, 1),
                            7: (PM_SUB_OP, dN, 0, 0),
                            5: (PM_ADD_OP, dS, 2, 1),
                            6: (PM_SUB_OP, dS, 0, 0),
                        }.items():
                            nc.vector._custom_dve(
                                op,
                                out=view(ydiag[k][:, osl][:, 0:W], W, 2, W),
                                in0=view(din[:, dcol : dcol + W], DW, 2, W),
                                in1=view(dE[:, eb + ecol : eb + ecol + W], WE, 2, W),
                                s0=float(DIR_W[k] * factors[k, ch] / (PM_EW_L / 2)),
                                s1=float(DIR_W[k] * biases[k, ch] / (PM_EW_L / 2)),
                            )

                    emit_window(0)
                    emit_window(1)
                    yield (sN, sS, sE, sW, ydiag)

            def accum_bands(t, img, tiles, bands):
                sN, sS, sE, sW, ydiag = tiles
                u = uA[img]
                # ---- TensorE: upd = 7u + sum_k y_k; ScalarE: u' = DT*upd ----
                for b in bands:
                    cl = b * WP + 2
                    sl = slice(b * W, (b + 1) * W)
                    upd = upd_pool.tile([128, W], F32, name="upd")
                    nc.tensor.matmul(upd[:], IDENT7[:], u[:, cl : cl + W],
                                     start=True, stop=False)
                    for s in (sN, sS, sE, sW):
                        nc.tensor.matmul(upd[:], T1_M[:], s[:, sl],
                                         start=False, stop=False)
                    for i, k in enumerate((4, 5, 6, 7)):
                        nc.tensor.matmul(upd[:], IL2_M[:], ydiag[k][:, sl],
                                         start=False, stop=(i == 3))
                    if t < NUM_ITER - 1:
                        nc.scalar.activation(
                            u[:, cl : cl + W], upd[:], ACT_COPY, scale=float(DT),
                        )
                    else:
                        so = io_pool.tile([128, W], F32, tag="stage_out", name="so")
                        nc.scalar.activation(so[:], upd[:], ACT_COPY, scale=float(DT))
                        nc.sync.dma_start(
                            o_d[img, b * 128 : (b + 1) * 128, :], so[:]
                        )

            # Software pipeline: emit image i's accumulation interleaved into
            # image i+1's eval (half between the two windows) so each engine's
            # in-order stream always has independent work behind a stall.
            for t in range(NUM_ITER):
                pending = None
                for img in range(IMGS):
                    tiles = next(eval_img(t, img))
                    if pending is not None:
                        accum_bands(t, img - 1, pending, range(BANDS))
                    pending = tiles
                accum_bands(t, IMGS - 1, pending, range(BANDS))

    nc.finalize()
    return nc


def _install_ntff_hook():
    """The agent image's antenv lacks axon_hooks; recreate it so trace=True works."""
    import types

    try:
        from antenv.axon_hooks import get_axon_ntff_profile_hook  # noqa: F401

        return
    except ImportError:
        pass
    import antenv

    mod = types.ModuleType("antenv.axon_hooks")
    _state = {"hook": None}
    mod.set_axon_ntff_profile_hook = lambda h: _state.__setitem__("hook", h)
    mod.get_axon_ntff_profile_hook = lambda: _state["hook"]
    sys.modules["antenv.axon_hooks"] = mod
    antenv.axon_hooks = mod
    so_path = "/opt/axon/libaxon_pjrt.so"
    if os.path.exists(so_path):
        sys.path.insert(0, "/root/.axon_site")
        try:
            from trn_agent_boot.trn_boot import _ntff_profile_via_ctypes

            hook = _ntff_profile_via_ctypes(so_path)
            if hook is not None:
                mod.set_axon_ntff_profile_hook(hook)
        except Exception as e:
            print(f"ntff hook install failed: {e}")


_CACHE = {}


def _get_nc(biases, factors):
    key = (biases.tobytes(), factors.tobytes())
    if key not in _CACHE:
        _CACHE[key] = build_nc(biases, factors)
    return _CACHE[key]


def kernel(x, biases, factors, _trace=False):
    x = np.ascontiguousarray(np.asarray(x, np.float32))
    biases = np.asarray(biases, np.float32)
    factors = np.asarray(factors, np.float32)
    nc = _get_nc(biases, factors)
    if _trace:
        _install_ntff_hook()

    wmat = _weight_mats()
    per_core = B // N_CORES
    in_maps = [
        {
            "x": x[i * per_core : (i + 1) * per_core].reshape(IMGS, H, W),
            "wmat": wmat,
        }
        for i in range(N_CORES)
    ]
    res = run_bass_kernel_spmd(nc, in_maps, core_ids=list(range(N_CORES)), trace=_trace)
    out = np.concatenate(
        [res.results[i]["out"].reshape(per_core, C, H, W) for i in range(N_CORES)],
        axis=0,
    )
    if _trace:
        kernel.last_exec_time_ns = res.exec_time_ns
        kernel.last_results = res
    return out
